# revision 4
# baseline (speedup 1.0000x reference)
"""InterpretableMultiHeadAttention on 8 Trainium2 NeuronCores (Bass/Tile).

Sharding: core c -> batch b = c//2, head-group hg = c%2 (8 of 16 heads).
Math folding (exact up to fp rounding):
  v' = v @ Wv.T + bv, x = sum_h attn_h @ v'_h, out = x @ Wo.T + bo
  Since softmax rows sum to 1:  attn @ (1 bv^T) = 1 bv^T, so
  out = (sum_h attn_h @ v_h) @ (Wo @ Wv).T + (H * Wo @ bv + bo)
The 1/sqrt(d) score scale folds into Wq/bq.

v2 schedule (per core):
- Q/K projections emitted as [128,1024] chunks; pair p+1's 4 chunks are
  burst-interleaved into pair p's jc loop during iq0 (no Scalar gaps).
- Per jc: fp16 scores pair at PE row-strips (0,0)/(64,0) into one
  [128,1024] PSUM tile; exp split between ScalarE (true Exp -> f16) and
  DVE (Schraudolph bit-trick: round(1477.32*s + 15316) as int16 viewed
  as f16, ~3% max rel err, cancels through the softmax ratio); PV fp16.
- Ones-columns at 64+(h%2) of each head's vv tile put softmax
  denominators on PSUM rows 64/65 of yA/yB.
- Per-pair finalize: Act copies yA/yB->f16, DVE adds den rows
  (cross-base 64->0), fast-approx reciprocal, f16 DRAM-bounce broadcast,
  f16 4x-mode divide+accumulate into y16. Out-projection matmuls DMA
  PSUM directly to DRAM.
Host sums the two partial projections per batch and adds the bias.
"""
import numpy as np

N_OUT = 1024
N_HEADS = 16
D_K = 64
B = 4
S = 2048
FC = 8          # 1024 contraction f-chunks of 128 (projections)
PAIRS = 4       # 8 local heads as 4 row-packed pairs
NMM = 512       # matmul moving free dim
JC = S // 128   # key chunks of 128
IQ = S // NMM   # query blocks of 512
MV = 72         # PV lhsT width: 64 v dims + ones cols at 64/65

A16 = 1024.0 / float(np.log(2.0))   # schraudolph f16 scale
B16 = 15.0 * 1024.0 - 44.0          # schraudolph f16 offset (round mode)

# exp engine assignment: which jc of 16 go to DVE (rest ScalarE)
DVE_JC = {2, 5, 8, 11, 13, 15}          # 6/16 in steady state
DVE_JC_IQ0 = {5, 13}                    # lighter DVE load while proj runs

_CACHE = {}


def _build_nc():
    from contextlib import ExitStack
    import concourse.bass as bass
    import concourse.bacc as bacc
    import concourse.tile as tile
    import concourse.mybir as mybir

    f16 = mybir.dt.float16
    f32 = mybir.dt.float32
    i16 = mybir.dt.int16
    Exp = mybir.ActivationFunctionType.Exp
    Copy = mybir.ActivationFunctionType.Copy
    AOp = mybir.AluOpType

    nc = bacc.Bacc("TRN2", target_bir_lowering=False, debug=False, num_devices=8)

    xq_d = nc.dram_tensor("xq", [FC, 128, S], f16, kind="ExternalInput")
    xk_d = nc.dram_tensor("xk", [FC, 128, S], f16, kind="ExternalInput")
    wq_d = nc.dram_tensor("wq", [128, FC, 512], f16, kind="ExternalInput")
    wk_d = nc.dram_tensor("wk", [128, FC, 512], f16, kind="ExternalInput")
    bq_d = nc.dram_tensor("bq", [128, PAIRS], f32, kind="ExternalInput")
    bk_d = nc.dram_tensor("bk", [128, PAIRS], f32, kind="ExternalInput")
    vv_d = nc.dram_tensor("vv", [PAIRS, 128, JC, 2, MV], f16, kind="ExternalInput")
    wov_d = nc.dram_tensor("wov", [64, N_OUT], f16, kind="ExternalInput")
    out_d = nc.dram_tensor("outT", [8, 128, S], f16, kind="ExternalOutput")
    den_d = nc.dram_tensor("den_scratch", [IQ, PAIRS, 2, NMM], f16)  # bounce

    with tile.TileContext(nc) as tc, ExitStack() as ctx:
        const = ctx.enter_context(tc.tile_pool(name="const", bufs=1))
        qkall = ctx.enter_context(tc.tile_pool(name="qkall", bufs=1))
        epool = ctx.enter_context(tc.tile_pool(name="epool", bufs=3))
        fin = ctx.enter_context(tc.tile_pool(name="fin", bufs=2))
        ostp = ctx.enter_context(tc.tile_pool(name="ostp", bufs=4))
        ps_s = ctx.enter_context(tc.tile_pool(name="ps_s", bufs=3, space="PSUM"))
        ps_y = ctx.enter_context(tc.tile_pool(name="ps_y", bufs=2, space="PSUM"))
        xctx = ExitStack()
        xstage = xctx.enter_context(tc.tile_pool(name="xstage", bufs=1))

        # ---- input loads (k-side first: proj k(p0) gates attention) ----
        wk_sb = const.tile([128, FC, 512], f16, tag="wk")
        nc.sync.dma_start(out=wk_sb[:], in_=wk_d[:])
        bk_sb = const.tile([128, PAIRS], f32, tag="bk")
        nc.sync.dma_start(out=bk_sb[:], in_=bk_d[:])
        bq_sb = const.tile([128, PAIRS], f32, tag="bq")
        nc.gpsimd.dma_start(out=bq_sb[:], in_=bq_d[:])
        wq_sb = const.tile([128, FC, 512], f16, tag="wq")
        nc.scalar.dma_start(out=wq_sb[:], in_=wq_d[:])
        engs = [nc.sync, nc.scalar, nc.gpsimd]
        xq_sb, xk_sb = [], []
        ei = 0
        for f in range(FC):
            t = xstage.tile([128, S], f16, tag=f"xk{f}")
            engs[ei % 3].dma_start(out=t[:], in_=xk_d[f])
            ei += 1
            xk_sb.append(t)
        for f in range(FC):
            t = xstage.tile([128, S], f16, tag=f"xq{f}")
            engs[ei % 3].dma_start(out=t[:], in_=xq_d[f])
            ei += 1
            xq_sb.append(t)
        vv_sb = []
        for p in range(PAIRS):
            t = qkall.tile([128, JC, 2, MV], f16, tag=f"vv{p}")
            nc.sync.dma_start(out=t[:], in_=vv_d[p])
            vv_sb.append(t)
        wov_sb = const.tile([64, N_OUT], f16, tag="wov")
        nc.sync.dma_start(out=wov_sb[:], in_=wov_d[:])

        qT, kT = {}, {}
        for p in range(PAIRS):
            qT[p] = qkall.tile([128, S], f16, tag=f"qT{p}", name=f"qT{p}")
            kT[p] = qkall.tile([128, S], f16, tag=f"kT{p}", name=f"kT{p}")

        def proj_chunk(p, which, ci):
            """One [128,1024] projection chunk: 16 matmuls + bias-add."""
            dst, w_sb, b_sb, x_sb = (
                (qT[p], wq_sb, bq_sb, xq_sb) if which == "q"
                else (kT[p], wk_sb, bk_sb, xk_sb))
            ps = ps_s.tile([128, 1024], f32, tag="mm")
            for hf in range(2):
                c0 = ci * 1024 + hf * 512
                for f in range(FC):
                    nc.tensor.matmul(
                        out=ps[:, hf * 512:(hf + 1) * 512],
                        lhsT=w_sb[:, f, p * 128:(p + 1) * 128],
                        rhs=x_sb[f][:, c0:c0 + 512],
                        start=(f == 0),
                        stop=(f == FC - 1),
                    )
            nc.vector.tensor_scalar(
                out=dst[:, ci * 1024:(ci + 1) * 1024],
                in0=ps[:],
                scalar1=b_sb[:, p:p + 1],
                scalar2=None,
                op0=AOp.add,
            )

        proj_pending = []
        for p in range(PAIRS):
            proj_pending.append((p, "k", 0))
            proj_pending.append((p, "k", 1))
            proj_pending.append((p, "q", 0))
            proj_pending.append((p, "q", 1))

        def emit_proj_burst():
            if proj_pending:
                proj_chunk(*proj_pending.pop(0))
            if not proj_pending:
                xctx.close()

        # upfront: p0 k both chunks + q chunk0
        proj_chunk(*proj_pending.pop(0))
        proj_chunk(*proj_pending.pop(0))
        proj_chunk(*proj_pending.pop(0))

        def finalize_stage1(iq, p, yA, yB):
            """Pair-end: copy y to f16, den -> recip -> f16 -> bounce bcast."""
            yA16 = fin.tile([MV, NMM], f16, tag="yA16")
            nc.scalar.activation(out=yA16[:], in_=yA[:], func=Copy)
            yB16 = fin.tile([MV, NMM], f16, tag="yB16")
            nc.scalar.activation(out=yB16[:], in_=yB[:], func=Copy)
            d2 = fin.tile([2, NMM], f32, tag="d2")
            nc.vector.tensor_tensor(out=d2[:], in0=yA16[64:66, :],
                                    in1=yB16[64:66, :], op=AOp.add)
            nc.vector.reciprocal_approx_fast(out=d2[:], in_=d2[:])
            r16 = fin.tile([2, NMM], f16, tag="r16")
            nc.scalar.activation(out=r16[:], in_=d2[:], func=Copy)
            nc.gpsimd.dma_start(out=den_d[iq, p], in_=r16[:])
            rb2 = fin.tile([64, 2, NMM], f16, tag="rb2")
            for g in range(2):
                row = den_d[iq, p, g:g + 1, :]
                bc = bass.AP(tensor=row.tensor, offset=row.offset,
                             ap=[[0, 64]] + row.ap[1:])
                nc.gpsimd.dma_start(out=rb2[:, g, :], in_=bc)
            return yA16, yB16, rb2

        def finalize_stage2(p, y16, st1):
            """Divide by den and accumulate heads into y16 (f16 4x mode)."""
            yA16, yB16, rb2 = st1
            if p == 0:
                nc.vector.tensor_tensor(out=y16[:], in0=yA16[0:64, :],
                                        in1=rb2[:, 0, :], op=AOp.mult)
            else:
                tmp = fin.tile([64, NMM], f16, tag="tmp")
                nc.vector.tensor_tensor(out=tmp[:], in0=yA16[0:64, :],
                                        in1=rb2[:, 0, :], op=AOp.mult)
                nc.vector.tensor_tensor(out=y16[:], in0=y16[:], in1=tmp[:],
                                        op=AOp.add)
            tmp = fin.tile([64, NMM], f16, tag="tmp")
            nc.vector.tensor_tensor(out=tmp[:], in0=yB16[0:64, :],
                                    in1=rb2[:, 1, :], op=AOp.mult)
            nc.vector.tensor_tensor(out=y16[:], in0=y16[:], in1=tmp[:],
                                    op=AOp.add)

        def outproj_half(iq, y16, half):
            i0 = iq * NMM
            for m in range(4 * half, 4 * half + 4):
                po = ps_s.tile([128, 1024], f32, tag="mm")
                nc.tensor.matmul(
                    out=po[:, :NMM],
                    lhsT=wov_sb[:, m * 128:(m + 1) * 128],
                    rhs=y16[:],
                    start=True, stop=True,
                )
                ost = ostp.tile([128, NMM], f16, tag="ost")
                if m % 2 == 0:
                    nc.scalar.activation(out=ost[:], in_=po[:, :NMM], func=Copy)
                else:
                    nc.vector.tensor_copy(out=ost[:], in_=po[:, :NMM])
                nc.sync.dma_start(out=out_d[m][:, i0:i0 + NMM], in_=ost[:])

        pend_st1 = None     # (p, stage1 tiles) awaiting divide+accumulate
        pend_out = None     # (iq, y16) awaiting output projection
        y16 = None
        for iq in range(IQ):
            i0 = iq * NMM
            dve_set = DVE_JC_IQ0 if iq == 0 else DVE_JC
            for p in range(PAIRS):
                yA = ps_y.tile([MV, NMM], f32, tag="yab")
                yB = ps_y.tile([MV, NMM], f32, tag="yab")
                for jc in range(JC):
                    j0 = jc * 128
                    sAB = ps_s.tile([128, 1024], f32, tag="mm")
                    nc.tensor.matmul(
                        out=sAB[:, :NMM],
                        lhsT=kT[p][0:64, j0:j0 + 128],
                        rhs=qT[p][0:64, i0:i0 + NMM],
                        start=True, stop=True,
                        tile_position=(0, 0),
                    )
                    nc.tensor.matmul(
                        out=sAB[:, NMM:],
                        lhsT=kT[p][64:128, j0:j0 + 128],
                        rhs=qT[p][64:128, i0:i0 + NMM],
                        start=True, stop=True,
                        tile_position=(64, 0),
                    )
                    eAB = epool.tile([128, 1024], f16, tag="e")
                    if jc in dve_set:
                        nc.vector.tensor_scalar(
                            out=eAB[:].bitcast(i16), in0=sAB[:],
                            scalar1=A16, scalar2=B16,
                            op0=AOp.mult, op1=AOp.add,
                        )
                    else:
                        nc.scalar.activation(out=eAB[:], in_=sAB[:], func=Exp)
                    nc.tensor.matmul(
                        out=yA[:],
                        lhsT=vv_sb[p][:, jc, 0, :],
                        rhs=eAB[:, :NMM],
                        start=(jc == 0), stop=(jc == JC - 1),
                        skip_group_check=True,
                    )
                    nc.tensor.matmul(
                        out=yB[:],
                        lhsT=vv_sb[p][:, jc, 1, :],
                        rhs=eAB[:, NMM:],
                        start=(jc == 0), stop=(jc == JC - 1),
                        skip_group_check=True,
                    )
                    if iq == 0 and jc in (3, 7, 11, 15):
                        emit_proj_burst()
                    if jc == 1 and pend_st1 is not None:
                        pp, st1 = pend_st1
                        if pp == 0:
                            y16 = fin.tile([64, NMM], f16, tag="y16")
                        finalize_stage2(pp, y16, st1)
                        pend_st1 = None
                        if pp == PAIRS - 1:
                            pend_out = (iq - 1, y16)
                    if jc == 5 and pend_out is not None:
                        outproj_half(pend_out[0], pend_out[1], 0)
                    if jc == 9 and pend_out is not None:
                        outproj_half(pend_out[0], pend_out[1], 1)
                        pend_out = None
                pend_st1 = (p, finalize_stage1(iq, p, yA, yB))
        # drain: last pair finalize + last block out-projection
        pp, st1 = pend_st1
        finalize_stage2(pp, y16, st1)
        outproj_half(IQ - 1, y16, 0)
        outproj_half(IQ - 1, y16, 1)

    nc.compile()
    return nc


def _prep(queries, keys, values, Wq, bq, Wk, bk, Wv, bv, Wo, bo):
    """Host-side sharding/layout prep. Returns (in_maps, bo_p)."""
    queries = np.asarray(queries, np.float32)
    keys = np.asarray(keys, np.float32)
    values = np.asarray(values, np.float32)
    Wq = np.asarray(Wq, np.float32)
    bq = np.asarray(bq, np.float32)
    Wk = np.asarray(Wk, np.float32)
    bk = np.asarray(bk, np.float32)
    Wv = np.asarray(Wv, np.float32)
    bv = np.asarray(bv, np.float32)
    Wo = np.asarray(Wo, np.float32)
    bo = np.asarray(bo, np.float32)

    scale = 1.0 / np.sqrt(np.float32(D_K))
    Wq_s = Wq * scale
    bq_s = bq * scale
    Wov = Wo @ Wv                       # [1024, 64]
    bo_p = bo + N_HEADS * (Wo @ bv)     # [1024]
    wov_h = np.ascontiguousarray(Wov.T.astype(np.float16))  # [64, 1024]

    in_maps = []
    for c in range(8):
        b = c // 2
        hg = c % 2
        hsl = slice(hg * 512, (hg + 1) * 512)
        xq = np.ascontiguousarray(
            queries[b].T.astype(np.float16).reshape(FC, 128, S))
        xk = np.ascontiguousarray(
            keys[b].T.astype(np.float16).reshape(FC, 128, S))
        wq = np.ascontiguousarray(
            Wq_s[hsl].T.astype(np.float16).reshape(FC, 128, 512).transpose(1, 0, 2))
        wk = np.ascontiguousarray(
            Wk[hsl].T.astype(np.float16).reshape(FC, 128, 512).transpose(1, 0, 2))
        bq_c = np.ascontiguousarray(bq_s[hsl].reshape(PAIRS, 128).T)
        bk_c = np.ascontiguousarray(bk[hsl].reshape(PAIRS, 128).T)
        vv = np.zeros((128, JC, 8, MV), np.float16)
        vv[:, :, :, :64] = (
            values[b][:, hsl].astype(np.float16)
            .reshape(JC, 128, 8, 64).transpose(1, 0, 2, 3))
        for h in range(8):
            vv[:, :, h, 64 + (h % 2)] = 1.0
        # [PAIRS, 128, JC, 2, MV] so each pair's slice is one contiguous DMA
        vv = vv.reshape(128, JC, PAIRS, 2, MV).transpose(2, 0, 1, 3, 4)
        in_maps.append({
            "xq": xq, "xk": xk, "wq": wq, "wk": wk,
            "bq": bq_c, "bk": bk_c, "vv": np.ascontiguousarray(vv),
            "wov": wov_h,
        })
    return in_maps, bo_p


def _build_in_maps(inputs):
    return _prep(**inputs)[0]


def _gather(results, bo_p):
    out = np.empty((B, S, N_OUT), np.float32)
    for b in range(B):
        oT = (np.asarray(results[2 * b]["outT"], np.float32)
              + np.asarray(results[2 * b + 1]["outT"], np.float32))
        out[b] = oT.reshape(N_OUT, S).T + bo_p
    return out


def kernel(queries, keys, values, Wq, bq, Wk, bk, Wv, bv, Wo, bo):
    from concourse.bass_utils import run_bass_kernel_spmd

    in_maps, bo_p = _prep(queries, keys, values, Wq, bq, Wk, bk, Wv, bv, Wo, bo)
    if "nc" not in _CACHE:
        _CACHE["nc"] = _build_nc()
    res = run_bass_kernel_spmd(_CACHE["nc"], in_maps, core_ids=list(range(8)))
    return _gather(res.results, bo_p)


# revision 5
# speedup vs baseline: 1.0061x; 1.0061x over previous
"""InterpretableMultiHeadAttention on 8 Trainium2 NeuronCores (Bass/Tile).

Sharding: core c -> batch b = c//2, head-group hg = c%2 (8 of 16 heads).
Math folding (exact up to fp rounding):
  v' = v @ Wv.T + bv, x = sum_h attn_h @ v'_h, out = x @ Wo.T + bo
  Since softmax rows sum to 1:  attn @ (1 bv^T) = 1 bv^T, so
  out = (sum_h attn_h @ v_h) @ (Wo @ Wv).T + (H * Wo @ bv + bo)
The 1/sqrt(d) score scale folds into Wq/bq.

v2 schedule (per core):
- Q/K projections emitted as [128,1024] chunks; pair p+1's 4 chunks are
  burst-interleaved into pair p's jc loop during iq0 (no Scalar gaps).
- Per jc: fp16 scores pair at PE row-strips (0,0)/(64,0) into one
  [128,1024] PSUM tile; exp split between ScalarE (true Exp -> f16) and
  DVE (Schraudolph bit-trick: round(1477.32*s + 15316) as int16 viewed
  as f16, ~3% max rel err, cancels through the softmax ratio); PV fp16.
- Ones-columns at 64+(h%2) of each head's vv tile put softmax
  denominators on PSUM rows 64/65 of yA/yB.
- Per-pair finalize: Act copies yA/yB->f16, DVE adds den rows
  (cross-base 64->0), fast-approx reciprocal, f16 DRAM-bounce broadcast,
  f16 4x-mode divide+accumulate into y16. Out-projection matmuls DMA
  PSUM directly to DRAM.
Host sums the two partial projections per batch and adds the bias.
"""
import numpy as np
import ml_dtypes

E4 = ml_dtypes.float8_e4m3

N_OUT = 1024
N_HEADS = 16
D_K = 64
B = 4
S = 2048
FC = 8          # 1024 contraction f-chunks of 128 (projections)
PAIRS = 4       # 8 local heads as 4 row-packed pairs
NMM = 512       # matmul moving free dim
JC = S // 128   # key chunks of 128
IQ = S // NMM   # query blocks of 512
MV = 72         # PV lhsT width: 64 v dims + ones cols at 64/65

A8 = 8.0 / float(np.log(2.0))   # schraudolph fp8e4 scale
B8 = 56.0 - 0.344               # schraudolph fp8e4 offset (round mode)

# exp engine assignment: which jc of 16 go to DVE (rest ScalarE).
# jc14/15 on DVE so ScalarE is free at pair end for the y copies.
DVE_JC = {2, 5, 8, 11, 14, 15}
DVE_JC_IQ0 = {5, 14}                    # lighter DVE load while proj runs

_CACHE = {}


def _build_nc():
    from contextlib import ExitStack
    import concourse.bass as bass
    import concourse.bacc as bacc
    import concourse.tile as tile
    import concourse.mybir as mybir

    f16 = mybir.dt.float16
    f32 = mybir.dt.float32
    i8 = mybir.dt.int8
    e4 = mybir.dt.float8e4
    Exp = mybir.ActivationFunctionType.Exp
    Copy = mybir.ActivationFunctionType.Copy
    AOp = mybir.AluOpType

    nc = bacc.Bacc("TRN2", target_bir_lowering=False, debug=False, num_devices=8)

    xq_d = nc.dram_tensor("xq", [FC, 128, S], f16, kind="ExternalInput")
    xk_d = nc.dram_tensor("xk", [FC, 128, S], f16, kind="ExternalInput")
    wq_d = nc.dram_tensor("wq", [128, FC, 512], f16, kind="ExternalInput")
    wk_d = nc.dram_tensor("wk", [128, FC, 512], f16, kind="ExternalInput")
    bq_d = nc.dram_tensor("bq", [128, PAIRS], f32, kind="ExternalInput")
    bk_d = nc.dram_tensor("bk", [128, PAIRS], f32, kind="ExternalInput")
    vv_d = nc.dram_tensor("vv", [PAIRS, 128, JC, 2, MV], e4, kind="ExternalInput")
    wov_d = nc.dram_tensor("wov", [64, N_OUT], f16, kind="ExternalInput")
    out_d = nc.dram_tensor("outT", [8, 128, S], f16, kind="ExternalOutput")
    den_d = nc.dram_tensor("den_scratch", [IQ, PAIRS, 2, NMM], f16)  # bounce

    with tile.TileContext(nc) as tc, ExitStack() as ctx:
        const = ctx.enter_context(tc.tile_pool(name="const", bufs=1))
        qkall = ctx.enter_context(tc.tile_pool(name="qkall", bufs=1))
        epool = ctx.enter_context(tc.tile_pool(name="epool", bufs=3))
        fin = ctx.enter_context(tc.tile_pool(name="fin", bufs=2))
        ostp = ctx.enter_context(tc.tile_pool(name="ostp", bufs=4))
        ps_s = ctx.enter_context(tc.tile_pool(name="ps_s", bufs=3, space="PSUM"))
        ps_y = ctx.enter_context(tc.tile_pool(name="ps_y", bufs=2, space="PSUM"))
        xctx = ExitStack()
        xstage = xctx.enter_context(tc.tile_pool(name="xstage", bufs=1))

        # ---- input loads (k-side first: proj k(p0) gates attention) ----
        wk_sb = const.tile([128, FC, 512], f16, tag="wk")
        nc.sync.dma_start(out=wk_sb[:], in_=wk_d[:])
        bk_sb = const.tile([128, PAIRS], f32, tag="bk")
        nc.sync.dma_start(out=bk_sb[:], in_=bk_d[:])
        bq_sb = const.tile([128, PAIRS], f32, tag="bq")
        nc.gpsimd.dma_start(out=bq_sb[:], in_=bq_d[:])
        wq_sb = const.tile([128, FC, 512], f16, tag="wq")
        nc.scalar.dma_start(out=wq_sb[:], in_=wq_d[:])
        engs = [nc.sync, nc.scalar, nc.gpsimd]
        xq_sb, xk_sb = [], []
        ei = 0
        for f in range(FC):
            t = xstage.tile([128, S], f16, tag=f"xk{f}")
            xk_sb.append(t)
        for f in range(FC):
            t = xstage.tile([128, S], f16, tag=f"xq{f}")
            xq_sb.append(t)
        # column-half loads: k half0 gates the first projection chunk
        for half in range(2):
            cs = slice(half * 1024, half * 1024 + 1024)
            for f in range(FC):
                engs[ei % 3].dma_start(out=xk_sb[f][:, cs], in_=xk_d[f][:, cs])
                ei += 1
        vv_sb = []
        for p in range(PAIRS):
            t = qkall.tile([128, JC, 2, MV], e4, tag=f"vv{p}")
            vv_sb.append(t)
        nc.sync.dma_start(out=vv_sb[0][:], in_=vv_d[0])
        for half in range(2):
            cs = slice(half * 1024, half * 1024 + 1024)
            for f in range(FC):
                engs[ei % 3].dma_start(out=xq_sb[f][:, cs], in_=xq_d[f][:, cs])
                ei += 1
        for p in range(1, PAIRS):
            nc.sync.dma_start(out=vv_sb[p][:], in_=vv_d[p])
        wov_sb = const.tile([64, N_OUT], f16, tag="wov")
        nc.sync.dma_start(out=wov_sb[:], in_=wov_d[:])

        qT, kT = {}, {}
        for p in range(PAIRS):
            qT[p] = qkall.tile([128, S], f16, tag=f"qT{p}", name=f"qT{p}")
            kT[p] = qkall.tile([128, S], f16, tag=f"kT{p}", name=f"kT{p}")

        def proj_chunk(p, which, ci):
            """One [128,1024] projection chunk: 16 matmuls + bias-add."""
            dst, w_sb, b_sb, x_sb = (
                (qT[p], wq_sb, bq_sb, xq_sb) if which == "q"
                else (kT[p], wk_sb, bk_sb, xk_sb))
            ps = ps_s.tile([128, 1024], f32, tag="mm")
            for hf in range(2):
                c0 = ci * 1024 + hf * 512
                for f in range(FC):
                    nc.tensor.matmul(
                        out=ps[:, hf * 512:(hf + 1) * 512],
                        lhsT=w_sb[:, f, p * 128:(p + 1) * 128],
                        rhs=x_sb[f][:, c0:c0 + 512],
                        start=(f == 0),
                        stop=(f == FC - 1),
                    )
            nc.vector.tensor_scalar(
                out=dst[:, ci * 1024:(ci + 1) * 1024],
                in0=ps[:],
                scalar1=b_sb[:, p:p + 1],
                scalar2=None,
                op0=AOp.add,
            )

        proj_pending = []
        for p in range(PAIRS):
            proj_pending.append((p, "k", 0))
            proj_pending.append((p, "k", 1))
            proj_pending.append((p, "q", 0))
            proj_pending.append((p, "q", 1))

        def emit_proj_burst():
            if proj_pending:
                proj_chunk(*proj_pending.pop(0))
            if not proj_pending:
                xctx.close()

        # upfront: p0 k both chunks + q chunk0
        proj_chunk(*proj_pending.pop(0))
        proj_chunk(*proj_pending.pop(0))
        proj_chunk(*proj_pending.pop(0))

        def finalize_stage1(iq, p, yA, yB):
            """Pair-end: copy y to f16, den -> recip -> f16 -> bounce bcast."""
            yA16 = fin.tile([MV, NMM], f16, tag="yA16")
            nc.scalar.activation(out=yA16[:], in_=yA[:], func=Copy)
            yB16 = fin.tile([MV, NMM], f16, tag="yB16")
            nc.scalar.activation(out=yB16[:], in_=yB[:], func=Copy)
            d2 = fin.tile([2, NMM], f32, tag="d2")
            nc.vector.tensor_tensor(out=d2[:], in0=yA16[64:66, :],
                                    in1=yB16[64:66, :], op=AOp.add)
            nc.vector.reciprocal_approx_fast(out=d2[:], in_=d2[:])
            r16 = fin.tile([2, NMM], f16, tag="r16")
            nc.scalar.activation(out=r16[:], in_=d2[:], func=Copy)
            nc.gpsimd.dma_start(out=den_d[iq, p], in_=r16[:])
            rb2 = fin.tile([64, 2, NMM], f16, tag="rb2")
            for g in range(2):
                row = den_d[iq, p, g:g + 1, :]
                bc = bass.AP(tensor=row.tensor, offset=row.offset,
                             ap=[[0, 64]] + row.ap[1:])
                nc.gpsimd.dma_start(out=rb2[:, g, :], in_=bc)
            return yA16, yB16, rb2

        def finalize_stage2(p, y16, st1):
            """Divide by den and accumulate heads into y16 (f16 4x mode)."""
            yA16, yB16, rb2 = st1
            if p == 0:
                nc.vector.tensor_tensor(out=y16[:], in0=yA16[0:64, :],
                                        in1=rb2[:, 0, :], op=AOp.mult)
            else:
                tmp = fin.tile([64, NMM], f16, tag="tmp")
                nc.vector.tensor_tensor(out=tmp[:], in0=yA16[0:64, :],
                                        in1=rb2[:, 0, :], op=AOp.mult)
                nc.vector.tensor_tensor(out=y16[:], in0=y16[:], in1=tmp[:],
                                        op=AOp.add)
            tmp = fin.tile([64, NMM], f16, tag="tmp")
            nc.vector.tensor_tensor(out=tmp[:], in0=yB16[0:64, :],
                                    in1=rb2[:, 1, :], op=AOp.mult)
            nc.vector.tensor_tensor(out=y16[:], in0=y16[:], in1=tmp[:],
                                    op=AOp.add)

        def outproj_half(iq, y16, half):
            i0 = iq * NMM
            for m in range(4 * half, 4 * half + 4):
                po = ps_s.tile([128, 1024], f32, tag="mm")
                nc.tensor.matmul(
                    out=po[:, :NMM],
                    lhsT=wov_sb[:, m * 128:(m + 1) * 128],
                    rhs=y16[:],
                    start=True, stop=True,
                )
                ost = ostp.tile([128, NMM], f16, tag="ost")
                if m % 2 == 0:
                    nc.scalar.activation(out=ost[:], in_=po[:, :NMM], func=Copy)
                else:
                    nc.vector.tensor_copy(out=ost[:], in_=po[:, :NMM])
                nc.sync.dma_start(out=out_d[m][:, i0:i0 + NMM], in_=ost[:])

        pend_st1 = None     # (p, stage1 tiles) awaiting divide+accumulate
        pend_out = None     # (iq, y16) awaiting output projection
        y16 = None
        for iq in range(IQ):
            i0 = iq * NMM
            dve_set = DVE_JC_IQ0 if iq == 0 else DVE_JC
            for p in range(PAIRS):
                yA = ps_y.tile([MV, NMM], f32, tag="yab")
                yB = ps_y.tile([MV, NMM], f32, tag="yab")
                for jc in range(JC):
                    j0 = jc * 128
                    sAB = ps_s.tile([128, 1024], f32, tag="mm")
                    nc.tensor.matmul(
                        out=sAB[:, :NMM],
                        lhsT=kT[p][0:64, j0:j0 + 128],
                        rhs=qT[p][0:64, i0:i0 + NMM],
                        start=True, stop=True,
                        tile_position=(0, 0),
                    )
                    nc.tensor.matmul(
                        out=sAB[:, NMM:],
                        lhsT=kT[p][64:128, j0:j0 + 128],
                        rhs=qT[p][64:128, i0:i0 + NMM],
                        start=True, stop=True,
                        tile_position=(64, 0),
                    )
                    eAB = epool.tile([128, 1024], e4, tag="e")
                    if jc in dve_set:
                        nc.vector.tensor_scalar(
                            out=eAB[:].bitcast(i8), in0=sAB[:],
                            scalar1=A8, scalar2=B8,
                            op0=AOp.mult, op1=AOp.add,
                        )
                    else:
                        nc.scalar.activation(out=eAB[:], in_=sAB[:], func=Exp)
                    nc.tensor.matmul(
                        out=yA[:],
                        lhsT=vv_sb[p][:, jc, 0, :],
                        rhs=eAB[:, :NMM],
                        start=(jc == 0), stop=(jc == JC - 1),
                        skip_group_check=True,
                    )
                    nc.tensor.matmul(
                        out=yB[:],
                        lhsT=vv_sb[p][:, jc, 1, :],
                        rhs=eAB[:, NMM:],
                        start=(jc == 0), stop=(jc == JC - 1),
                        skip_group_check=True,
                    )
                    if iq == 0 and jc in (3, 7, 11, 15):
                        emit_proj_burst()
                    if jc == 1 and pend_st1 is not None:
                        pp, st1 = pend_st1
                        if pp == 0:
                            y16 = fin.tile([64, NMM], f16, tag="y16")
                        finalize_stage2(pp, y16, st1)
                        pend_st1 = None
                        if pp == PAIRS - 1:
                            pend_out = (iq - 1, y16)
                    if jc == 8 and pend_out is not None:
                        outproj_half(pend_out[0], pend_out[1], 0)
                    if jc == 12 and pend_out is not None:
                        outproj_half(pend_out[0], pend_out[1], 1)
                        pend_out = None
                pend_st1 = (p, finalize_stage1(iq, p, yA, yB))
        # drain: last pair finalize + last block out-projection
        pp, st1 = pend_st1
        finalize_stage2(pp, y16, st1)
        outproj_half(IQ - 1, y16, 0)
        outproj_half(IQ - 1, y16, 1)

    nc.compile()
    return nc


def _prep(queries, keys, values, Wq, bq, Wk, bk, Wv, bv, Wo, bo):
    """Host-side sharding/layout prep. Returns (in_maps, bo_p)."""
    queries = np.asarray(queries, np.float32)
    keys = np.asarray(keys, np.float32)
    values = np.asarray(values, np.float32)
    Wq = np.asarray(Wq, np.float32)
    bq = np.asarray(bq, np.float32)
    Wk = np.asarray(Wk, np.float32)
    bk = np.asarray(bk, np.float32)
    Wv = np.asarray(Wv, np.float32)
    bv = np.asarray(bv, np.float32)
    Wo = np.asarray(Wo, np.float32)
    bo = np.asarray(bo, np.float32)

    scale = 1.0 / np.sqrt(np.float32(D_K))
    Wq_s = Wq * scale
    bq_s = bq * scale
    Wov = Wo @ Wv                       # [1024, 64]
    bo_p = bo + N_HEADS * (Wo @ bv)     # [1024]
    wov_h = np.ascontiguousarray(Wov.T.astype(np.float16))  # [64, 1024]

    in_maps = []
    for c in range(8):
        b = c // 2
        hg = c % 2
        hsl = slice(hg * 512, (hg + 1) * 512)
        xq = np.ascontiguousarray(
            queries[b].T.astype(np.float16).reshape(FC, 128, S))
        xk = np.ascontiguousarray(
            keys[b].T.astype(np.float16).reshape(FC, 128, S))
        wq = np.ascontiguousarray(
            Wq_s[hsl].T.astype(np.float16).reshape(FC, 128, 512).transpose(1, 0, 2))
        wk = np.ascontiguousarray(
            Wk[hsl].T.astype(np.float16).reshape(FC, 128, 512).transpose(1, 0, 2))
        bq_c = np.ascontiguousarray(bq_s[hsl].reshape(PAIRS, 128).T)
        bk_c = np.ascontiguousarray(bk[hsl].reshape(PAIRS, 128).T)
        vv = np.zeros((128, JC, 8, MV), np.float32)
        vv[:, :, :, :64] = (
            values[b][:, hsl]
            .reshape(JC, 128, 8, 64).transpose(1, 0, 2, 3))
        for h in range(8):
            vv[:, :, h, 64 + (h % 2)] = 1.0
        # [PAIRS, 128, JC, 2, MV] so each pair's slice is one contiguous DMA
        vv = vv.reshape(128, JC, PAIRS, 2, MV).transpose(2, 0, 1, 3, 4)
        vv = vv.astype(E4)
        in_maps.append({
            "xq": xq, "xk": xk, "wq": wq, "wk": wk,
            "bq": bq_c, "bk": bk_c, "vv": np.ascontiguousarray(vv),
            "wov": wov_h,
        })
    return in_maps, bo_p


def _build_in_maps(inputs):
    return _prep(**inputs)[0]


def _gather(results, bo_p):
    out = np.empty((B, S, N_OUT), np.float32)
    for b in range(B):
        oT = (np.asarray(results[2 * b]["outT"], np.float32)
              + np.asarray(results[2 * b + 1]["outT"], np.float32))
        out[b] = oT.reshape(N_OUT, S).T + bo_p
    return out


def kernel(queries, keys, values, Wq, bq, Wk, bk, Wv, bv, Wo, bo):
    from concourse.bass_utils import run_bass_kernel_spmd

    in_maps, bo_p = _prep(queries, keys, values, Wq, bq, Wk, bk, Wv, bv, Wo, bo)
    if "nc" not in _CACHE:
        _CACHE["nc"] = _build_nc()
    res = run_bass_kernel_spmd(_CACHE["nc"], in_maps, core_ids=list(range(8)))
    return _gather(res.results, bo_p)


# revision 7
# speedup vs baseline: 1.1071x; 1.1003x over previous
"""InterpretableMultiHeadAttention on 8 Trainium2 NeuronCores (Bass/Tile).

Sharding: core c -> batch b = c//2, head-group hg = c%2 (8 of 16 heads).
Math folding (exact up to fp rounding):
  v' = v @ Wv.T + bv, x = sum_h attn_h @ v'_h, out = x @ Wo.T + bo
  Since softmax rows sum to 1:  attn @ (1 bv^T) = 1 bv^T, so
  out = (sum_h attn_h @ v_h) @ (Wo @ Wv).T + (H * Wo @ bv + bo)
The 1/sqrt(d) score scale folds into Wq/bq.

v2 schedule (per core):
- Q/K projections emitted as [128,1024] chunks; pair p+1's 4 chunks are
  burst-interleaved into pair p's jc loop during iq0 (no Scalar gaps).
- Per jc: fp16 scores pair at PE row-strips (0,0)/(64,0) into one
  [128,1024] PSUM tile; exp split between ScalarE (true Exp -> f16) and
  DVE (Schraudolph bit-trick: round(1477.32*s + 15316) as int16 viewed
  as f16, ~3% max rel err, cancels through the softmax ratio); PV fp16.
- Ones-columns at 64+(h%2) of each head's vv tile put softmax
  denominators on PSUM rows 64/65 of yA/yB.
- Per-pair finalize: Act copies yA/yB->f16, DVE adds den rows
  (cross-base 64->0), fast-approx reciprocal, f16 DRAM-bounce broadcast,
  f16 4x-mode divide+accumulate into y16. Out-projection matmuls DMA
  PSUM directly to DRAM.
Host sums the two partial projections per batch and adds the bias.
"""
import numpy as np
import ml_dtypes

E4 = ml_dtypes.float8_e4m3

N_OUT = 1024
N_HEADS = 16
D_K = 64
B = 4
S = 2048
FC = 8          # 1024 contraction f-chunks of 128 (projections)
PAIRS = 4       # 8 local heads as 4 row-packed pairs
NMM = 512       # matmul moving free dim
JC = S // 128   # key chunks of 128
IQ = S // NMM   # query blocks of 512
MV = 72         # PV lhsT width: 64 v dims + ones cols at 64/65

A8 = 8.0 / float(np.log(2.0))   # schraudolph fp8e4 scale
B8 = 56.0 - 0.344               # schraudolph fp8e4 offset (round mode)

# exp engine assignment: which jc of 16 go to DVE (rest ScalarE).
# jc14/15 on DVE so ScalarE is free at pair end for the y copies.
DVE_JC = {2, 5, 8, 11, 14, 15}
DVE_JC_IQ0 = {5, 14}                    # lighter DVE load while proj runs

_CACHE = {}


def _build_nc():
    from contextlib import ExitStack
    import concourse.bass as bass
    import concourse.bacc as bacc
    import concourse.tile as tile
    import concourse.mybir as mybir

    f16 = mybir.dt.float16
    f32 = mybir.dt.float32
    i8 = mybir.dt.int8
    e4 = mybir.dt.float8e4
    Exp = mybir.ActivationFunctionType.Exp
    Copy = mybir.ActivationFunctionType.Copy
    AOp = mybir.AluOpType

    nc = bacc.Bacc("TRN2", target_bir_lowering=False, debug=False, num_devices=8)

    xq_d = nc.dram_tensor("xq", [FC, 128, S], f16, kind="ExternalInput")
    xk_d = nc.dram_tensor("xk", [FC, 128, S], f16, kind="ExternalInput")
    wq_d = nc.dram_tensor("wq", [128, FC, 512], f16, kind="ExternalInput")
    wk_d = nc.dram_tensor("wk", [128, FC, 512], f16, kind="ExternalInput")
    bq_d = nc.dram_tensor("bq", [128, PAIRS], f32, kind="ExternalInput")
    bk_d = nc.dram_tensor("bk", [128, PAIRS], f32, kind="ExternalInput")
    vv_d = nc.dram_tensor("vv", [PAIRS, 128, JC, 2, MV], e4, kind="ExternalInput")
    wov_d = nc.dram_tensor("wov", [64, N_OUT], f16, kind="ExternalInput")
    out_d = nc.dram_tensor("outT", [8, 128, S], f16, kind="ExternalOutput")
    den_d = nc.dram_tensor("den_scratch", [IQ, PAIRS, 2, NMM], f16)  # bounce

    with tile.TileContext(nc) as tc, ExitStack() as ctx:
        const = ctx.enter_context(tc.tile_pool(name="const", bufs=1))
        qkall = ctx.enter_context(tc.tile_pool(name="qkall", bufs=1))
        epool = ctx.enter_context(tc.tile_pool(name="epool", bufs=3))
        fin = ctx.enter_context(tc.tile_pool(name="fin", bufs=2))
        ostp = ctx.enter_context(tc.tile_pool(name="ostp", bufs=4))
        ps_s = ctx.enter_context(tc.tile_pool(name="ps_s", bufs=3, space="PSUM"))
        ps_y = ctx.enter_context(tc.tile_pool(name="ps_y", bufs=2, space="PSUM"))
        xctx = ExitStack()
        xstage = xctx.enter_context(tc.tile_pool(name="xstage", bufs=1))

        # ---- input loads (k-side first: proj k(p0) gates attention) ----
        wk_sb = const.tile([128, FC, 512], f16, tag="wk")
        nc.sync.dma_start(out=wk_sb[:], in_=wk_d[:])
        bk_sb = const.tile([128, PAIRS], f32, tag="bk")
        nc.sync.dma_start(out=bk_sb[:], in_=bk_d[:])
        bq_sb = const.tile([128, PAIRS], f32, tag="bq")
        nc.gpsimd.dma_start(out=bq_sb[:], in_=bq_d[:])
        wq_sb = const.tile([128, FC, 512], f16, tag="wq")
        nc.scalar.dma_start(out=wq_sb[:], in_=wq_d[:])
        engs = [nc.sync, nc.scalar, nc.gpsimd]
        xq_sb, xk_sb = [], []
        ei = 0
        for f in range(FC):
            t = xstage.tile([128, S], f16, tag=f"xk{f}")
            xk_sb.append(t)
        for f in range(FC):
            t = xstage.tile([128, S], f16, tag=f"xq{f}")
            xq_sb.append(t)
        # column-half loads: k half0 gates the first projection chunk
        for half in range(2):
            cs = slice(half * 1024, half * 1024 + 1024)
            for f in range(FC):
                engs[ei % 3].dma_start(out=xk_sb[f][:, cs], in_=xk_d[f][:, cs])
                ei += 1
        vv_sb = []
        for p in range(PAIRS):
            t = qkall.tile([128, JC, 2, MV], e4, tag=f"vv{p}")
            vv_sb.append(t)
        nc.sync.dma_start(out=vv_sb[0][:], in_=vv_d[0])
        for half in range(2):
            cs = slice(half * 1024, half * 1024 + 1024)
            for f in range(FC):
                engs[ei % 3].dma_start(out=xq_sb[f][:, cs], in_=xq_d[f][:, cs])
                ei += 1
        for p in range(1, PAIRS):
            nc.sync.dma_start(out=vv_sb[p][:], in_=vv_d[p])
        wov_sb = const.tile([64, N_OUT], f16, tag="wov")
        nc.sync.dma_start(out=wov_sb[:], in_=wov_d[:])

        qT, kT = {}, {}
        for p in range(PAIRS):
            qT[p] = qkall.tile([128, S], f16, tag=f"qT{p}", name=f"qT{p}")
            kT[p] = qkall.tile([128, S], f16, tag=f"kT{p}", name=f"kT{p}")

        def proj_chunk(p, which, ci):
            """One [128,1024] projection chunk: 16 matmuls + bias-add."""
            dst, w_sb, b_sb, x_sb = (
                (qT[p], wq_sb, bq_sb, xq_sb) if which == "q"
                else (kT[p], wk_sb, bk_sb, xk_sb))
            ps = ps_s.tile([128, 1024], f32, tag="mm")
            for hf in range(2):
                c0 = ci * 1024 + hf * 512
                for f in range(FC):
                    nc.tensor.matmul(
                        out=ps[:, hf * 512:(hf + 1) * 512],
                        lhsT=w_sb[:, f, p * 128:(p + 1) * 128],
                        rhs=x_sb[f][:, c0:c0 + 512],
                        start=(f == 0),
                        stop=(f == FC - 1),
                    )
            nc.vector.tensor_scalar(
                out=dst[:, ci * 1024:(ci + 1) * 1024],
                in0=ps[:],
                scalar1=b_sb[:, p:p + 1],
                scalar2=None,
                op0=AOp.add,
            )

        proj_pending = []
        for p in range(PAIRS):
            proj_pending.append((p, "k", 0))
            proj_pending.append((p, "k", 1))
            proj_pending.append((p, "q", 0))
            proj_pending.append((p, "q", 1))

        def emit_proj_burst():
            if proj_pending:
                proj_chunk(*proj_pending.pop(0))
            if not proj_pending:
                xctx.close()

        # upfront: p0 k both chunks + q chunk0
        proj_chunk(*proj_pending.pop(0))
        proj_chunk(*proj_pending.pop(0))
        proj_chunk(*proj_pending.pop(0))

        def finalize_stage1(iq, p, yA, yB):
            """Pair-end: copy y to f16, den -> recip -> f16 -> bounce bcast."""
            yA16 = fin.tile([MV, NMM], f16, tag="yA16")
            nc.scalar.activation(out=yA16[:], in_=yA[:], func=Copy)
            yB16 = fin.tile([MV, NMM], f16, tag="yB16")
            nc.scalar.activation(out=yB16[:], in_=yB[:], func=Copy)
            d2 = fin.tile([2, NMM], f32, tag="d2")
            nc.vector.tensor_tensor(out=d2[:], in0=yA16[64:66, :],
                                    in1=yB16[64:66, :], op=AOp.add)
            nc.vector.reciprocal_approx_fast(out=d2[:], in_=d2[:])
            r16 = fin.tile([2, NMM], f16, tag="r16")
            nc.scalar.activation(out=r16[:], in_=d2[:], func=Copy)
            nc.gpsimd.dma_start(out=den_d[iq, p], in_=r16[:])
            rb2 = fin.tile([64, 2, NMM], f16, tag="rb2")
            for g in range(2):
                row = den_d[iq, p, g:g + 1, :]
                bc = bass.AP(tensor=row.tensor, offset=row.offset,
                             ap=[[0, 64]] + row.ap[1:])
                nc.gpsimd.dma_start(out=rb2[:, g, :], in_=bc)
            return yA16, yB16, rb2

        def finalize_stage2(p, y16, st1):
            """Divide by den and accumulate heads into y16 (f16 4x mode)."""
            yA16, yB16, rb2 = st1
            if p == 0:
                nc.vector.tensor_tensor(out=y16[:], in0=yA16[0:64, :],
                                        in1=rb2[:, 0, :], op=AOp.mult)
            else:
                tmp = fin.tile([64, NMM], f16, tag="tmp")
                nc.vector.tensor_tensor(out=tmp[:], in0=yA16[0:64, :],
                                        in1=rb2[:, 0, :], op=AOp.mult)
                nc.vector.tensor_tensor(out=y16[:], in0=y16[:], in1=tmp[:],
                                        op=AOp.add)
            tmp = fin.tile([64, NMM], f16, tag="tmp")
            nc.vector.tensor_tensor(out=tmp[:], in0=yB16[0:64, :],
                                    in1=rb2[:, 1, :], op=AOp.mult)
            nc.vector.tensor_tensor(out=y16[:], in0=y16[:], in1=tmp[:],
                                    op=AOp.add)

        def outproj_half(iq, y16, half):
            i0 = iq * NMM
            for m in range(4 * half, 4 * half + 4):
                po = ps_s.tile([128, 1024], f32, tag="mm")
                nc.tensor.matmul(
                    out=po[:, :NMM],
                    lhsT=wov_sb[:, m * 128:(m + 1) * 128],
                    rhs=y16[:],
                    start=True, stop=True,
                )
                ost = ostp.tile([128, NMM], f16, tag="ost")
                if m % 2 == 0:
                    nc.scalar.activation(out=ost[:], in_=po[:, :NMM], func=Copy)
                else:
                    nc.vector.tensor_copy(out=ost[:], in_=po[:, :NMM])
                nc.sync.dma_start(out=out_d[m][:, i0:i0 + NMM], in_=ost[:])

        # ---- flattened software-pipelined stream over all (iq, p, jc) ----
        DEPTH = 2                     # PV trails scores/exp by 2 items
        items = [(iq, p, jc) for iq in range(IQ) for p in range(PAIRS)
                 for jc in range(JC)]
        idx_of = {t: i for i, t in enumerate(items)}
        ytiles = {}                   # (iq,p) -> (yA, yB)
        st1_of = {}                   # (iq,p) -> stage1 tiles
        y16_of = {}                   # iq -> y16 tile
        extras = {}                   # index -> list of callables

        def add_extra(i, fn):
            extras.setdefault(i, []).append(fn)

        for iq in range(IQ):
            for p in range(PAIRS):
                i15 = idx_of[(iq, p, 15)]
                s1i = i15 + DEPTH + 1          # right after PV(jc15) emitted
                def mk_s1(iq=iq, p=p):
                    def fn():
                        yA, yB = ytiles.pop((iq, p))
                        st1_of[(iq, p)] = finalize_stage1(iq, p, yA, yB)
                    return fn
                add_extra(s1i, mk_s1())
                def mk_s2(iq=iq, p=p):
                    def fn():
                        if p == 0:
                            y16_of[iq] = fin.tile([64, NMM], f16, tag="y16", name=f"y16_{iq}")
                        finalize_stage2(p, y16_of[iq], st1_of.pop((iq, p)))
                    return fn
                add_extra(s1i + 4, mk_s2())
                if p == PAIRS - 1:
                    def mk_op(iq=iq, half=0):
                        def fn():
                            outproj_half(iq, y16_of[iq], half)
                        return fn
                    add_extra(s1i + 7, mk_op(iq, 0))
                    add_extra(s1i + 10, mk_op(iq, 1))

        def emit_pv(it):
            iq, p, jc = it
            i0 = iq * NMM
            if jc == 0:
                yA = ps_y.tile([MV, NMM], f32, tag="yab")
                yB = ps_y.tile([MV, NMM], f32, tag="yab")
                ytiles[(iq, p)] = (yA, yB)
            yA, yB = ytiles[(iq, p)]
            eAB = etiles.pop(it)
            nc.tensor.matmul(
                out=yA[:],
                lhsT=vv_sb[p][:, jc, 0, :],
                rhs=eAB[:, :NMM],
                start=(jc == 0), stop=(jc == JC - 1),
                skip_group_check=True,
            )
            nc.tensor.matmul(
                out=yB[:],
                lhsT=vv_sb[p][:, jc, 1, :],
                rhs=eAB[:, NMM:],
                start=(jc == 0), stop=(jc == JC - 1),
                skip_group_check=True,
            )

        etiles = {}
        n_items = len(items)
        for i in range(n_items + DEPTH):
            if i < n_items:
                iq, p, jc = items[i]
                i0 = iq * NMM
                j0 = jc * 128
                sAB = ps_s.tile([128, 1024], f32, tag="mm")
                nc.tensor.matmul(
                    out=sAB[:, :NMM],
                    lhsT=kT[p][0:64, j0:j0 + 128],
                    rhs=qT[p][0:64, i0:i0 + NMM],
                    start=True, stop=True,
                    tile_position=(0, 0),
                )
                nc.tensor.matmul(
                    out=sAB[:, NMM:],
                    lhsT=kT[p][64:128, j0:j0 + 128],
                    rhs=qT[p][64:128, i0:i0 + NMM],
                    start=True, stop=True,
                    tile_position=(64, 0),
                )
                eAB = epool.tile([128, 1024], e4, tag="e")
                dve_set = DVE_JC_IQ0 if iq == 0 else DVE_JC
                if jc in dve_set:
                    nc.vector.tensor_scalar(
                        out=eAB[:].bitcast(i8), in0=sAB[:],
                        scalar1=A8, scalar2=B8,
                        op0=AOp.mult, op1=AOp.add,
                    )
                else:
                    nc.scalar.activation(out=eAB[:], in_=sAB[:], func=Exp)
                etiles[items[i]] = eAB
                if iq == 0 and jc in (3, 7, 11, 15):
                    emit_proj_burst()
            for fn in extras.pop(i, ()):
                fn()
            if i >= DEPTH:
                emit_pv(items[i - DEPTH])
        for fn in sorted((k, v) for k, v in extras.items()):
            for f in fn[1]:
                f()

    nc.compile()
    return nc


def _prep(queries, keys, values, Wq, bq, Wk, bk, Wv, bv, Wo, bo):
    """Host-side sharding/layout prep. Returns (in_maps, bo_p)."""
    queries = np.asarray(queries, np.float32)
    keys = np.asarray(keys, np.float32)
    values = np.asarray(values, np.float32)
    Wq = np.asarray(Wq, np.float32)
    bq = np.asarray(bq, np.float32)
    Wk = np.asarray(Wk, np.float32)
    bk = np.asarray(bk, np.float32)
    Wv = np.asarray(Wv, np.float32)
    bv = np.asarray(bv, np.float32)
    Wo = np.asarray(Wo, np.float32)
    bo = np.asarray(bo, np.float32)

    scale = 1.0 / np.sqrt(np.float32(D_K))
    Wq_s = Wq * scale
    bq_s = bq * scale
    Wov = Wo @ Wv                       # [1024, 64]
    bo_p = bo + N_HEADS * (Wo @ bv)     # [1024]
    wov_h = np.ascontiguousarray(Wov.T.astype(np.float16))  # [64, 1024]

    in_maps = []
    for c in range(8):
        b = c // 2
        hg = c % 2
        hsl = slice(hg * 512, (hg + 1) * 512)
        xq = np.ascontiguousarray(
            queries[b].T.astype(np.float16).reshape(FC, 128, S))
        xk = np.ascontiguousarray(
            keys[b].T.astype(np.float16).reshape(FC, 128, S))
        wq = np.ascontiguousarray(
            Wq_s[hsl].T.astype(np.float16).reshape(FC, 128, 512).transpose(1, 0, 2))
        wk = np.ascontiguousarray(
            Wk[hsl].T.astype(np.float16).reshape(FC, 128, 512).transpose(1, 0, 2))
        bq_c = np.ascontiguousarray(bq_s[hsl].reshape(PAIRS, 128).T)
        bk_c = np.ascontiguousarray(bk[hsl].reshape(PAIRS, 128).T)
        vv = np.zeros((128, JC, 8, MV), np.float32)
        vv[:, :, :, :64] = (
            values[b][:, hsl]
            .reshape(JC, 128, 8, 64).transpose(1, 0, 2, 3))
        for h in range(8):
            vv[:, :, h, 64 + (h % 2)] = 1.0
        # [PAIRS, 128, JC, 2, MV] so each pair's slice is one contiguous DMA
        vv = vv.reshape(128, JC, PAIRS, 2, MV).transpose(2, 0, 1, 3, 4)
        vv = vv.astype(E4)
        in_maps.append({
            "xq": xq, "xk": xk, "wq": wq, "wk": wk,
            "bq": bq_c, "bk": bk_c, "vv": np.ascontiguousarray(vv),
            "wov": wov_h,
        })
    return in_maps, bo_p


def _build_in_maps(inputs):
    return _prep(**inputs)[0]


def _gather(results, bo_p):
    out = np.empty((B, S, N_OUT), np.float32)
    for b in range(B):
        oT = (np.asarray(results[2 * b]["outT"], np.float32)
              + np.asarray(results[2 * b + 1]["outT"], np.float32))
        out[b] = oT.reshape(N_OUT, S).T + bo_p
    return out


def kernel(queries, keys, values, Wq, bq, Wk, bk, Wv, bv, Wo, bo):
    from concourse.bass_utils import run_bass_kernel_spmd

    in_maps, bo_p = _prep(queries, keys, values, Wq, bq, Wk, bk, Wv, bv, Wo, bo)
    if "nc" not in _CACHE:
        _CACHE["nc"] = _build_nc()
    res = run_bass_kernel_spmd(_CACHE["nc"], in_maps, core_ids=list(range(8)))
    return _gather(res.results, bo_p)
